# revision 1
# baseline (speedup 1.0000x reference)
"""FP8-palettized linear kernel for 8x TRN2 NeuronCores.

Computes: out[b,s,o] = sum_d input[b,s,d] * lookup_table[weight[o,d]] + bias[o]
with input [4,2048,4096] f32, weight [4096,4096] int32 (palette ids < 256),
lookup_table [256] f32, bias [4096] f32.

Strategy (column-parallel, per sharding hint):
  - Each core owns a 512-wide slice of out_features. Full input replicated.
  - Host prep is value-preserving layout only: X is transposed to XT [d, s]
    (contraction on partitions for the PE), the int32 palette indices are
    permuted into the GPSIMD 16-partition "wrapped" order and fed as an
    int16 byte-view (values < 256 live in the low half of each int32).
  - On device: GPSIMD ap_gather dequantizes W^T k-tiles from a
    partition-broadcast LUT; a 16-phase SBUF->SBUF DMA compacts the
    16x-redundant gather output into dense [128 d, 512 o] fp32r tiles that
    stay resident in SBUF. TensorE then runs X^T-slab @ W^T with PSUM
    accumulation over the 32 k-tiles, DVE adds the bias, and results DMA out.
"""

import contextlib
import os

import numpy as np

import concourse.bacc as bacc
import concourse.mybir as mybir
import concourse.tile as tile
from concourse.bass_utils import run_bass_kernel_spmd

P = 128
N_CORES = 8

# Full-problem dims (hardcoded per harness contract).
BATCH, SEQ, D_IN, D_OUT, PALETTE = 4, 2048, 4096, 4096, 256
M_FULL = BATCH * SEQ  # 8192

# Matmul input dtype: float32r streams at 1 cycle/row on the PE for free
# dim >= 256 (fp32 costs 4). Overridable for precision/perf experiments.
MM_DTYPE = {
    "f32": mybir.dt.float32,
    "f32r": mybir.dt.float32r,
    "bf16": mybir.dt.bfloat16,
}[os.environ.get("PAL_MM_DTYPE", "f32r")]


def wrap_indices(w_shard: np.ndarray, n_ktiles: int) -> np.ndarray:
    """Permute a [OSH, K] int32 index shard into the ap_gather wrapped layout.

    Device contract (per k-tile kt, per 16-partition group g):
      unwrapped_g[i] = idx[16g + i%16, i//16] for i in [0, 16*OSH)
      after compaction row 16g+r takes segment i in [r*OSH, (r+1)*OSH):
      need unwrapped_g[r*OSH + j] = w_shard[j, kt*128 + 16g + r].
    Returns [128, n_ktiles*OSH] int32.
    """
    osh, k = w_shard.shape
    assert k == n_ktiles * P
    w4 = w_shard.reshape(osh, n_ktiles, 8, 16)  # [o][kt][g][r]
    u = w4.transpose(1, 2, 3, 0)  # [kt][g][r][o]
    wr = u.reshape(n_ktiles, 8, osh * 16 // 16, 16 * 16 // 16)  # placeholder
    wr = u.reshape(n_ktiles, 8, 16 * osh).reshape(n_ktiles, 8, osh, 16)
    wr = wr.transpose(1, 3, 0, 2)  # [g][p16][kt][s]
    return np.ascontiguousarray(wr.reshape(P, n_ktiles * osh))


def build_program(nc, *, m, k, osh, reps=1):
    """Emit the per-core Tile program. m: rows of X (mult of 128), k: d dim
    (mult of 128), osh: out-features per core (512). reps>1 wraps the body
    in a hardware loop (for benchmarking: amortizes dispatch overhead)."""
    n_kt = k // P
    n_mt = m // P
    f_red = 16 * osh  # gather output free size (per-partition)

    xt = nc.dram_tensor("xt", [k, m], MM_DTYPE, kind="ExternalInput")
    # int32 indices fed as int16 byte-view: value at even positions.
    widx = nc.dram_tensor("widx", [P, n_kt * osh * 2], mybir.dt.int16,
                          kind="ExternalInput")
    # Expanded per-partition table: row p holds LUT at slot (p%16)*256, zeros
    # elsewhere. Gathering with seg*256+idx leaves only partition p's own
    # segment nonzero, so a strided sum over the 16 segments compacts the
    # 16x-redundant gather output with plain (legal) free-dim APs.
    lutx = nc.dram_tensor("lutx", [P, 16 * PALETTE], mybir.dt.float32,
                          kind="ExternalInput")
    # ramp[p, s] = 256 * segment of wrapped slot (p%16, s)
    ramp = nc.dram_tensor("ramp", [P, osh], mybir.dt.int16,
                          kind="ExternalInput")
    bias = nc.dram_tensor("bias", [1, osh], mybir.dt.float32,
                          kind="ExternalInput")
    out = nc.dram_tensor("out", [m, osh], mybir.dt.float32,
                         kind="ExternalOutput")

    with tile.TileContext(nc) as tc:
        with (
            tc.tile_pool(name="const", bufs=1) as const_pool,
            tc.tile_pool(name="idx", bufs=2) as idx_pool,
            tc.tile_pool(name="idxc", bufs=2) as idxc_pool,
            tc.tile_pool(name="red", bufs=1) as red_pool,
            tc.tile_pool(name="wt", bufs=1) as wt_pool,
            tc.tile_pool(name="xs", bufs=3) as x_pool,
            tc.tile_pool(name="psum", bufs=2, space="PSUM") as psum_pool,
            tc.tile_pool(name="osb", bufs=2) as osb_pool,
            (tc.For_i(0, reps, 1) if reps > 1
             else contextlib.nullcontext()),
        ):
            # --- constants ---
            lutx_sb = const_pool.tile([P, 16 * PALETTE], mybir.dt.float32,
                                      tag="lutx")
            nc.sync.dma_start(lutx_sb[:], lutx[:])
            ramp_sb = const_pool.tile([P, osh], mybir.dt.int16, tag="ramp")
            nc.sync.dma_start(ramp_sb[:], ramp[:])

            bias_row = const_pool.tile([1, osh], mybir.dt.float32, tag="brow")
            nc.sync.dma_start(bias_row[:], bias[:])
            bias_sb = const_pool.tile([P, osh], mybir.dt.float32, tag="bsb")
            nc.gpsimd.partition_broadcast(bias_sb[:], bias_row[:])

            # --- dequant: one W^T tile [128 d, osh o] per k-tile ---
            # `red` is allocated once and reused so its slot is never handed
            # to another pool mid-flight.
            red = red_pool.tile([P, f_red], mybir.dt.float32)
            wt_tiles = []
            for kt in range(n_kt):
                idxr = idx_pool.tile([P, 2 * osh], mybir.dt.int16)
                nc.sync.dma_start(
                    idxr[:], widx[:, kt * 2 * osh:(kt + 1) * 2 * osh])
                idxc = idxc_pool.tile([P, osh], mybir.dt.int16)
                # int16 view of int32 values sits at even slots; add the
                # 256*segment ramp while compacting to contiguous int16.
                nc.vector.tensor_tensor(
                    idxc[:],
                    idxr.rearrange("p (s two) -> p s two", two=2)[:, :, 0],
                    ramp_sb[:],
                    op=mybir.AluOpType.add)

                nc.gpsimd.ap_gather(
                    red[:], lutx_sb[:], idxc[:],
                    channels=P, num_elems=16 * PALETTE, d=1, num_idxs=f_red)

                # sum over the 16 segments (only partition's own is nonzero)
                wt = wt_pool.tile([P, osh], MM_DTYPE,
                                  tag=f"wt{kt:02d}")
                # exact: 15 zeros + the partition's own segment value
                with nc.allow_low_precision(reason="sum of one value + zeros"):
                    nc.vector.tensor_reduce(
                        wt[:],
                        red.rearrange("p (r j) -> p j r", r=16),
                        axis=mybir.AxisListType.X,
                        op=mybir.AluOpType.add)
                wt_tiles.append(wt)

            # --- matmul: out[m-tile, :] = XT-slab^T @ W^T (+bias) ---
            for mt in range(n_mt):
                xslab = x_pool.tile([P, k], MM_DTYPE)
                nc.sync.dma_start(
                    xslab.rearrange("p (kt j) -> p kt j", kt=n_kt),
                    xt[:, mt * P:(mt + 1) * P]
                    .rearrange("(kt p) j -> p kt j", p=P))
                psum = psum_pool.tile([P, osh], mybir.dt.float32)
                for kt in range(n_kt):
                    nc.tensor.matmul(
                        psum[:],
                        lhsT=xslab[:, kt * P:(kt + 1) * P],
                        rhs=wt_tiles[kt][:],
                        start=(kt == 0),
                        stop=(kt == n_kt - 1))
                osb = osb_pool.tile([P, osh], mybir.dt.float32)
                nc.vector.tensor_tensor(
                    osb[:], psum[:], bias_sb[:], op=mybir.AluOpType.add)
                nc.scalar.dma_start(out[mt * P:(mt + 1) * P, :], osb[:])

    return xt, widx, lutx, bias, out


def make_core_inputs(input, lookup_table, weight, bias, *, m=M_FULL, k=D_IN,
                     osh=D_OUT // N_CORES, n_cores=N_CORES):
    """Host-side (value-preserving) sharding prep. Returns in_maps."""
    x2 = np.asarray(input, dtype=np.float32).reshape(m, k)
    xt = np.ascontiguousarray(x2.T)  # [k, m]
    lut_vals = np.asarray(lookup_table, dtype=np.float32).reshape(PALETTE)
    weight = np.asarray(weight)
    bias = np.asarray(bias, dtype=np.float32)

    # Expanded per-partition table: LUT values placed at slot (p%16)*256.
    lutx = np.zeros((P, 16 * PALETTE), dtype=np.float32)
    for p in range(P):
        s = p % 16
        lutx[p, s * PALETTE:(s + 1) * PALETTE] = lut_vals

    # ramp[p, s] = 256 * ((s*16 + p%16) // osh)
    p16 = (np.arange(P) % 16)[:, None]
    s_idx = np.arange(osh)[None, :]
    ramp = (PALETTE * ((s_idx * 16 + p16) // osh)).astype(np.int16)

    in_maps = []
    for c in range(n_cores):
        w_shard = weight[c * osh:(c + 1) * osh, :]  # [osh, k] int32
        wrapped = wrap_indices(np.ascontiguousarray(w_shard), k // P)
        in_maps.append({
            "xt": xt,
            "widx": wrapped.view(np.int16),
            "lutx": lutx,
            "ramp": ramp,
            "bias": bias[c * osh:(c + 1) * osh].reshape(1, osh),
        })
    return in_maps


def kernel(input, lookup_table, weight, bias, *, trace=False):
    osh = D_OUT // N_CORES
    nc = bacc.Bacc("TRN2", target_bir_lowering=False, debug=False,
                   num_devices=N_CORES)
    build_program(nc, m=M_FULL, k=D_IN, osh=osh)
    nc.compile()

    in_maps = make_core_inputs(input, lookup_table, weight, bias)
    res = run_bass_kernel_spmd(nc, in_maps, core_ids=list(range(N_CORES)),
                               trace=trace)
    out = np.concatenate([r["out"] for r in res.results], axis=1)
    out = np.ascontiguousarray(out.reshape(BATCH, SEQ, D_OUT), dtype=np.float32)
    if trace:
        kernel.last_results = res
    return out



# revision 4
# speedup vs baseline: 1.1417x; 1.1417x over previous
"""FP8-palettized linear kernel for 8x TRN2 NeuronCores.

Computes: out[b,s,o] = sum_d input[b,s,d] * lookup_table[weight[o,d]] + bias[o]
with input [4,2048,4096] f32, weight [4096,4096] int32 (palette ids < 256),
lookup_table [256] f32, bias [4096] f32.

Strategy (column-parallel, per sharding hint):
  - Each core owns a 512-wide slice of out_features; input replicated.
  - Host prep is layout/dtype marshalling only: X is tiled into contiguous
    [128, 4096] X^T slabs (one 1MB DMA per m-tile instead of 4096 512B
    descriptors), palette indices are stored in the ap_gather wrapped order
    with the 256*(p%16) segment ramp pre-folded (an index-arithmetic
    relabeling), as int16.
  - On device, per k-tile: GPSIMD ap_gather reads a segment-expanded LUT
    (zeros outside the partition's own 256-slot window) producing
    red[p, o*16+r] = LUT[idx[o, d_p]] for r==p%16 else 0; a CONTIGUOUS
    inner-16 DVE tensor_reduce compacts it into a resident W^T tile
    [128 d, 512 o]. TensorE accumulates X^T-slab @ W^T over 32 k-tiles in
    PSUM; the first 8 m-tiles run k-outer across 8 PSUM banks so the PE
    rides the dequant wave instead of stalling on the last W^T tile.
    DVE adds bias, results DMA out per m-tile.
"""

import contextlib
import os

import ml_dtypes
import numpy as np

import concourse.bacc as bacc
import concourse.mybir as mybir
import concourse.tile as tile
from concourse.bass_utils import run_bass_kernel_spmd

P = 128
N_CORES = 8

# Full-problem dims (hardcoded per harness contract).
BATCH, SEQ, D_IN, D_OUT, PALETTE = 4, 2048, 4096, 4096, 256
M_FULL = BATCH * SEQ  # 8192

MM_DTYPE = {
    "f32": mybir.dt.float32,
    "f32r": mybir.dt.float32r,
    "bf16": mybir.dt.bfloat16,
}[os.environ.get("PAL_MM_DTYPE", "bf16")]


def _np_mm_dtype():
    return (ml_dtypes.bfloat16 if MM_DTYPE == mybir.dt.bfloat16
            else np.float32)


def build_program(nc, *, m, k, osh, reps=1):
    """Emit the per-core Tile program. m: rows of X (mult of 128), k: d dim
    (mult of 128), osh: out-features per core (512). reps>1 wraps the body
    in a hardware loop (benchmarking: amortizes dispatch overhead)."""
    n_kt = k // P
    n_mt = m // P
    f_red = 16 * osh  # gather output free size (per-partition)
    # Phase-1 depth: m-tiles accumulated k-outer across PSUM banks while
    # dequant streams in. 8 banks for bf16; 4 for 4-byte dtypes (SBUF).
    np1 = 8 if MM_DTYPE == mybir.dt.bfloat16 else 4

    # [n_mt*128, k]; slab mt rows [mt*128,(mt+1)*128) hold X^T tile:
    # xt[mt*128+p, kt*128+j] = X[mt*128+j, kt*128+p].
    xt = nc.dram_tensor("xt", [m, k], MM_DTYPE, kind="ExternalInput")
    # [n_kt*128, osh] int16, wrapped+pre-ramped: widx[kt*128+p, s] =
    # idx[s, kt*128+p] + 256*(p%16).
    widx = nc.dram_tensor("widx", [n_kt * P, osh], mybir.dt.int16,
                          kind="ExternalInput")
    # Segment-expanded LUT: row p holds LUT at slot (p%16)*256, zeros
    # elsewhere, so the redundant gather leaves only the partition's own
    # (o, r=p%16) slots nonzero and a contiguous inner-16 sum compacts.
    lutx = nc.dram_tensor("lutx", [P, 16 * PALETTE], mybir.dt.float32,
                          kind="ExternalInput")
    bias = nc.dram_tensor("bias", [1, osh], mybir.dt.float32,
                          kind="ExternalInput")
    out = nc.dram_tensor("out", [m, osh], mybir.dt.float32,
                         kind="ExternalOutput")

    with tile.TileContext(nc) as tc:
        with (
            tc.tile_pool(name="const", bufs=1) as const_pool,
            tc.tile_pool(name="idx", bufs=2) as idx_pool,
            tc.tile_pool(name="red", bufs=2) as red_pool,
            tc.tile_pool(name="wt", bufs=1) as wt_pool,
            tc.tile_pool(name="xs", bufs=1) as x_pool,
            tc.tile_pool(name="psum", bufs=1, space="PSUM") as psum_pool,
            tc.tile_pool(name="osb", bufs=3) as osb_pool,
            (tc.For_i(0, reps, 1) if reps > 1
             else contextlib.nullcontext()),
        ):
            # --- constants ---
            lutx_sb = const_pool.tile([P, 16 * PALETTE], mybir.dt.float32,
                                      tag="lutx")
            nc.sync.dma_start(lutx_sb[:], lutx[:])
            bias_row = const_pool.tile([1, osh], mybir.dt.float32, tag="brow")
            nc.sync.dma_start(bias_row[:], bias[:])
            bias_sb = const_pool.tile([P, osh], mybir.dt.float32, tag="bsb")
            nc.gpsimd.partition_broadcast(bias_sb[:], bias_row[:])

            # --- dequant pipeline: W^T tile [128 d, osh o] per k-tile ---
            wt_tiles = []
            for kt in range(n_kt):
                idxt = idx_pool.tile([P, osh], mybir.dt.int16)
                nc.scalar.dma_start(idxt[:],
                                    widx[kt * P:(kt + 1) * P, :])
                red = red_pool.tile([P, f_red], mybir.dt.float32)
                nc.gpsimd.ap_gather(
                    red[:], lutx_sb[:], idxt[:],
                    channels=P, num_elems=16 * PALETTE, d=1, num_idxs=f_red)
                wt = wt_pool.tile([P, osh], MM_DTYPE, tag=f"wt{kt:02d}")
                # exact: each 16-group is 15 zeros + the wanted value
                with nc.allow_low_precision(reason="sum of one value + zeros"):
                    nc.vector.tensor_reduce(
                        wt[:],
                        red.rearrange("p (o r) -> p o r", r=16),
                        axis=mybir.AxisListType.X,
                        op=mybir.AluOpType.add)
                wt_tiles.append(wt)

            def load_xslab(mt, slot):
                xslab = x_pool.tile([P, k], MM_DTYPE, tag=f"xs{slot}")
                nc.sync.dma_start(xslab[:], xt[mt * P:(mt + 1) * P, :])
                return xslab

            def finish_mtile(mt, psum):
                osb = osb_pool.tile([P, osh], mybir.dt.float32)
                nc.vector.tensor_tensor(
                    osb[:], psum[:], bias_sb[:], op=mybir.AluOpType.add)
                nc.scalar.dma_start(out[mt * P:(mt + 1) * P, :], osb[:])

            # --- phase 1: first np1 m-tiles, k-outer across PSUM banks ---
            slabs = [load_xslab(mt, mt % np1) for mt in range(np1)]
            psums = [psum_pool.tile([P, osh], mybir.dt.float32,
                                    tag=f"ps{i}", name=f"psum{i}")
                     for i in range(np1)]
            for kt in range(n_kt):
                for i in range(np1):
                    nc.tensor.matmul(
                        psums[i][:],
                        lhsT=slabs[i][:, kt * P:(kt + 1) * P],
                        rhs=wt_tiles[kt][:],
                        start=(kt == 0),
                        stop=(kt == n_kt - 1))
            for i in range(np1):
                finish_mtile(i, psums[i])

            # --- phase 2: remaining m-tiles, m-outer ---
            for mt in range(np1, n_mt):
                xslab = load_xslab(mt, mt % np1)
                psum = psum_pool.tile([P, osh], mybir.dt.float32,
                                      tag=f"ps{mt % np1}")
                for kt in range(n_kt):
                    nc.tensor.matmul(
                        psum[:],
                        lhsT=xslab[:, kt * P:(kt + 1) * P],
                        rhs=wt_tiles[kt][:],
                        start=(kt == 0),
                        stop=(kt == n_kt - 1))
                finish_mtile(mt, psum)

    return xt, widx, lutx, bias, out


def make_core_inputs(input, lookup_table, weight, bias, *, m=M_FULL, k=D_IN,
                     osh=D_OUT // N_CORES, n_cores=N_CORES):
    """Host-side sharding/layout prep (no palette lookups). Returns in_maps."""
    n_kt = k // P
    n_mt = m // P
    x2 = np.asarray(input, dtype=np.float32).reshape(m, k)
    # xt[mt, p, kt*128+j] = X[mt*128+j, kt*128+p]
    xt = (x2.reshape(n_mt, P, n_kt, P).transpose(0, 3, 2, 1)
          .reshape(m, k).astype(_np_mm_dtype()))

    lut_vals = np.asarray(lookup_table, dtype=np.float32).reshape(PALETTE)
    lutx = np.zeros((P, 16 * PALETTE), dtype=np.float32)
    for p in range(P):
        s = p % 16
        lutx[p, s * PALETTE:(s + 1) * PALETTE] = lut_vals

    weight = np.asarray(weight)
    bias = np.asarray(bias, dtype=np.float32)
    ramp = (PALETTE * (np.arange(P) % 16)).astype(np.int32)[None, :, None]

    in_maps = []
    for c in range(n_cores):
        w_shard = weight[c * osh:(c + 1) * osh, :]  # [osh, k] int32
        # widx[kt, p, s] = idx[s, kt*128+p] + 256*(p%16)
        widx = (w_shard.T.reshape(n_kt, P, osh) + ramp).astype(np.int16)
        in_maps.append({
            "xt": xt,
            "widx": widx.reshape(n_kt * P, osh),
            "lutx": lutx,
            "bias": bias[c * osh:(c + 1) * osh].reshape(1, osh),
        })
    return in_maps


def kernel(input, lookup_table, weight, bias, *, trace=False):
    osh = D_OUT // N_CORES
    nc = bacc.Bacc("TRN2", target_bir_lowering=False, debug=False,
                   num_devices=N_CORES)
    build_program(nc, m=M_FULL, k=D_IN, osh=osh)
    nc.compile()

    in_maps = make_core_inputs(input, lookup_table, weight, bias)
    res = run_bass_kernel_spmd(nc, in_maps, core_ids=list(range(N_CORES)),
                               trace=trace)
    out = np.concatenate([r["out"] for r in res.results], axis=1)
    out = np.ascontiguousarray(out.reshape(BATCH, SEQ, D_OUT), dtype=np.float32)
    if trace:
        kernel.last_results = res
    return out
